# revision 4
# baseline (speedup 1.0000x reference)
"""Bidirectional GRU (AbstractBiRNN) Trainium2 Bass kernel.

Problem: B=32, T=512, D=U=512, fp32 in/out.
    outs_f = GRU_scan(x, Wf, Uf, bf)          # forward over t
    outs_b = GRU_scan(x[:, ::-1], Wb, Ub, bb) # backward (scan order kept)
    out = concat([outs_f, outs_b], axis=-1)   # [B, T, 2U]

Strategy (8 NeuronCores, zero inter-core communication):
  - core c: direction d = c//4 (0=fwd, 1=bwd), batch shard s = c%4 (rows 8s..8s+8).
  - Each core projects its own x shard (x @ W', fp32r matmuls at N=512, full rate)
    directly into SBUF in a gate-transposed layout, then runs the 512-step GRU
    scan locally.
  - Scan layout is "U-major folded": every per-step tensor lives in one
    [128 partitions, n_slices, 8 batch] SBUF tile (partition = U-dim within a
    128-slice). Matmuls keep recurrent weights stationary (lhsT = U_rec block
    [128,128]) and stream h as the N=8 moving operand, so gate outputs come out
    already U-major and no per-step transpose is ever needed.
  - hard_sigmoid(v) = clip(0.2 v + 0.5, 0, 1) is folded into the weights:
    W'_zr = 0.2 Wzr, b'_zr = 0.2 bzr + 0.5, U'_zr = 0.2 Uzr, so per step
    z,r = clip(psum + xg', 0, 1) -- two fused DVE ops.
  - Projection of chunk ts+1 is interleaved between scan steps of chunk ts to
    fill PE idle slots.

Host folds/reassembles layouts; all hot-loop compute is on-device.
"""

import os
import numpy as np
import ml_dtypes

import concourse.bass as bass
import concourse.tile as tile
from concourse import bacc, mybir
from concourse.bass_utils import run_bass_kernel_spmd

F32 = mybir.dt.float32
F32R = mybir.dt.float32r
BF16 = mybir.dt.bfloat16
AF = mybir.ActivationFunctionType
OP = mybir.AluOpType

B, T, D, U = 32, 512, 512, 512
NCORE = 8
BP = B // 4          # batch rows per core (4 shards per direction) = 8
CHUNK = 64           # scan steps per xg chunk resident in SBUF
KD = D // 128        # k-slices of contraction (4)
GZ = (2 * U) // 128  # zr gate slices (8)
GH = U // 128        # cand gate slices (4)
G = GZ + GH          # total gate slices (12)

# Scan matmul dtype: bf16 => fast weight loads (FWL); fp32 => exact, ~2x slower
SCAN_BF16 = os.environ.get("GRU_SCAN_DT", "bf16") == "bf16"
SDT = BF16 if SCAN_BF16 else F32
SDT_NP = ml_dtypes.bfloat16 if SCAN_BF16 else np.float32


def _build(t_steps=T, reps=1):
    """Emit the SPMD program (identical for all cores; data differs)."""
    nch = t_steps // CHUNK
    ntok = BP * t_steps

    nc = bacc.Bacc("TRN2", target_bir_lowering=False, debug=False,
                   num_devices=NCORE)

    # DRAM I/O (per core). xT[p, k, tau] = x[b, t, 128k+p], tau = t*BP + b.
    xT_d = nc.dram_tensor("xT", [128, KD, ntok], F32R, kind="ExternalInput").ap()
    wp_d = nc.dram_tensor("Wp", [128, KD, G * 128], F32R, kind="ExternalInput").ap()
    bias_d = nc.dram_tensor("bias", [128, G], F32, kind="ExternalInput").ap()
    uzr_d = nc.dram_tensor("Uzr", [128, KD, GZ * 128], SDT, kind="ExternalInput").ap()
    uh_d = nc.dram_tensor("Uh", [128, KD, GH * 128], SDT, kind="ExternalInput").ap()
    # outT[ts, p, t_in, s, b] = h_{ts*CHUNK+t_in}[b, 128s+p]
    out_d = nc.dram_tensor("outT", [nch, 128, CHUNK, GH, BP], F32,
                           kind="ExternalOutput").ap()

    with tile.TileContext(nc) as tc:
        with (
            tc.tile_pool(name="singles", bufs=1) as singles,
            tc.tile_pool(name="chunks", bufs=2) as chunks,
            tc.tile_pool(name="outs", bufs=2) as outs,
            tc.tile_pool(name="step", bufs=3) as stepp,
            tc.tile_pool(name="ps_zr", bufs=2, space="PSUM") as ps_zr,
            tc.tile_pool(name="ps_c", bufs=2, space="PSUM") as ps_c,
            tc.tile_pool(name="ps_p", bufs=2, space="PSUM") as ps_p,
        ):
            # ---- resident tensors ----
            xT = singles.tile([128, KD, ntok], F32R)
            wp = singles.tile([128, KD, G * 128], F32R)
            bias = singles.tile([128, G], F32)
            uzr = singles.tile([128, KD, GZ * 128], SDT)
            uh = singles.tile([128, KD, GH * 128], SDT)
            nc.sync.dma_start(out=xT, in_=xT_d)
            nc.sync.dma_start(out=wp, in_=wp_d)
            nc.sync.dma_start(out=bias, in_=bias_d)
            nc.sync.dma_start(out=uzr, in_=uzr_d)
            nc.sync.dma_start(out=uh, in_=uh_d)

            h0_f = singles.tile([128, GH, BP], F32)
            nc.vector.memset(h0_f, 0.0)
            h0_m = h0_f
            if SCAN_BF16:
                h0_m = singles.tile([128, GH, BP], BF16)
                nc.vector.memset(h0_m, 0.0)

            def emit_proj(ts, g, ct):
                """xg'[:, g, ts-chunk] = x @ Wp[:, g-slice] + bias, into SBUF."""
                pp = ps_p.tile([128, CHUNK * BP], F32)
                for k in range(KD):
                    nc.tensor.matmul(
                        out=pp[:],
                        lhsT=wp[:, k, 128 * g:128 * (g + 1)],
                        rhs=xT[:, k, CHUNK * BP * ts:CHUNK * BP * (ts + 1)],
                        start=(k == 0), stop=(k == KD - 1))
                nc.scalar.activation(ct[:, g, :], pp[:], AF.Identity,
                                     bias=bias[:, g:g + 1], scale=1.0)

            def scan_step(ct, ot, t_in, hp_f, hp_m):
                """One GRU step. hp_f/hp_m: previous h (fp32 AP / matmul-dtype AP).
                Returns (h_f32_ap, h_mm_ap) for the next step."""
                zr_ps = ps_zr.tile([128, GZ, BP], F32)
                for m in range(GZ):
                    for k in range(KD):
                        nc.tensor.matmul(
                            out=zr_ps[:, m, :],
                            lhsT=uzr[:, k, 128 * m:128 * (m + 1)],
                            rhs=hp_m[:, k, :],
                            start=(k == 0), stop=(k == KD - 1))
                zr = stepp.tile([128, GZ, BP], F32)
                # z,r = clip(hzr' + xzr', 0, 1)
                nc.vector.scalar_tensor_tensor(
                    out=zr, in0=zr_ps, scalar=0.0,
                    in1=ct[:, 0:GZ, BP * t_in:BP * (t_in + 1)],
                    op0=OP.bypass, op1=OP.add)
                nc.vector.tensor_scalar(
                    out=zr, in0=zr, scalar1=1.0, scalar2=0.0,
                    op0=OP.min, op1=OP.max)
                rh = stepp.tile([128, GH, BP], SDT)
                nc.vector.tensor_tensor(rh, zr[:, GH:GZ, :], hp_f, op=OP.mult)
                # off the critical path: z*h and (1-z), run while MM2 streams
                zh = stepp.tile([128, GH, BP], F32)
                nc.vector.tensor_tensor(zh, zr[:, 0:GH, :], hp_f, op=OP.mult)
                omz = stepp.tile([128, GH, BP], F32)
                nc.vector.tensor_scalar(out=omz, in0=zr[:, 0:GH, :],
                                        scalar1=-1.0, scalar2=1.0,
                                        op0=OP.mult, op1=OP.add)
                c_ps = ps_c.tile([128, GH, BP], F32)
                for m in range(GH):
                    for k in range(KD):
                        nc.tensor.matmul(
                            out=c_ps[:, m, :],
                            lhsT=uh[:, k, 128 * m:128 * (m + 1)],
                            rhs=rh[:, k, :],
                            start=(k == 0), stop=(k == KD - 1))
                tmp = stepp.tile([128, GH, BP], F32)
                nc.vector.scalar_tensor_tensor(
                    out=tmp, in0=c_ps, scalar=0.0,
                    in1=ct[:, GZ:G, BP * t_in:BP * (t_in + 1)],
                    op0=OP.bypass, op1=OP.add)
                cand = stepp.tile([128, GH, BP], F32)
                nc.scalar.activation(cand, tmp, AF.Tanh)
                t2 = stepp.tile([128, GH, BP], F32)
                nc.vector.tensor_tensor(t2, omz, cand, op=OP.mult)
                hn = ot[:, t_in, :, :]
                nc.vector.tensor_tensor(hn, zh, t2, op=OP.add)
                if SCAN_BF16:
                    hb = stepp.tile([128, GH, BP], BF16)
                    nc.scalar.activation(hb, hn, AF.Copy)
                    return hn, hb
                return hn, hn

            for rep in range(reps):
                ct_cur = chunks.tile([128, G, CHUNK * BP], F32)
                for g in range(G):
                    emit_proj(0, g, ct_cur)
                hp_f, hp_m = h0_f, h0_m
                for ts in range(nch):
                    ot = outs.tile([128, CHUNK, GH, BP], F32)
                    pq = []
                    ct_next = None
                    if ts + 1 < nch:
                        ct_next = chunks.tile([128, G, CHUNK * BP], F32)
                        pq = [(ts + 1, g) for g in range(G)]
                    for t_in in range(CHUNK):
                        hp_f, hp_m = scan_step(ct_cur, ot, t_in, hp_f, hp_m)
                        if t_in % 5 == 2 and pq:
                            emit_proj(*pq.pop(0), ct_next)
                    nc.sync.dma_start(out=out_d[ts], in_=ot)
                    ct_cur = ct_next
                    # rep boundary: reset h to zeros for reproducible timing
                hp_f, hp_m = h0_f, h0_m

    nc.compile()
    return nc


_CACHE = {}


def _get_nc(t_steps=T, reps=1):
    key = (t_steps, reps, SCAN_BF16)
    if key not in _CACHE:
        _CACHE[key] = _build(t_steps, reps)
    return _CACHE[key]


def _prep_inputs(x, Wf, Uf, bf, Wb, Ub, bb, t_steps=T):
    """Build per-core in_maps (host-side fold of scales + layouts)."""
    x = np.asarray(x, dtype=np.float32)
    in_maps = []
    for c in range(NCORE):
        d, s = divmod(c, 4)
        W = np.asarray(Wf if d == 0 else Wb, np.float32)
        Urec = np.asarray(Uf if d == 0 else Ub, np.float32)
        bvec = np.asarray(bf if d == 0 else bb, np.float32)
        rows = slice(BP * s, BP * (s + 1))
        xr = x[rows, :t_steps, :]                     # [BP, t, D]
        if d == 1:
            xr = xr[:, ::-1, :]
        # xT[p, k, tau] = xr[b, t, 128k+p]
        xT = np.ascontiguousarray(
            xr.transpose(2, 1, 0).reshape(KD, 128, t_steps * BP)
              .transpose(1, 0, 2))
        # folded projection weights / bias (hard_sigmoid affine into zr part)
        Wp = W.copy()
        Wp[:, :2 * U] *= 0.2
        bp = bvec.copy()
        bp = np.concatenate([0.2 * bp[:2 * U] + 0.5, bp[2 * U:]])
        WpT = np.ascontiguousarray(
            Wp.reshape(KD, 128, G * 128).transpose(1, 0, 2))
        biasT = np.ascontiguousarray(
            bp.reshape(G, 128).transpose(1, 0))
        Uzr = np.ascontiguousarray(
            (0.2 * Urec[:, :2 * U]).reshape(KD, 128, GZ * 128)
            .transpose(1, 0, 2)).astype(SDT_NP)
        Uh = np.ascontiguousarray(
            Urec[:, 2 * U:].reshape(KD, 128, GH * 128)
            .transpose(1, 0, 2)).astype(SDT_NP)
        in_maps.append({
            "xT": xT.astype(np.float32),
            "Wp": WpT.astype(np.float32),
            "bias": biasT.astype(np.float32),
            "Uzr": Uzr,
            "Uh": Uh,
        })
    return in_maps


def _assemble(results, t_steps=T):
    out = np.empty((B, t_steps, 2 * U), np.float32)
    for c in range(NCORE):
        d, s = divmod(c, 4)
        arr = results[c]["outT"]                      # [nch,128,CHUNK,GH,BP]
        blk = arr.transpose(4, 0, 2, 3, 1).reshape(BP, t_steps, U)
        out[BP * s:BP * (s + 1), :, d * U:(d + 1) * U] = blk
    return out


def kernel(x, Wf, Uf, bf, Wb, Ub, bb):
    nc = _get_nc()
    in_maps = _prep_inputs(x, Wf, Uf, bf, Wb, Ub, bb)
    res = run_bass_kernel_spmd(nc, in_maps, core_ids=list(range(NCORE)))
    return _assemble(res.results)
